# revision 31
# baseline (speedup 1.0000x reference)
"""Trainium2 Bass kernel for nn_AttentionFusion (dense_transformer).

Reference computation per batch element b (B=8 -> one NeuronCore each):
    w_ds = bilinear_downsample(feat_wide[b], 4)   # [C,64,64], exact 2x2 avg at (4i+1..4i+2)
    n_ds = bilinear_downsample(feat_narrow[b], 4)
    Q = w_ds.reshape(C, N); K = n_ds.reshape(C, N)    # N = 4096
    attn = softmax(Q^T K / sqrt(C), axis=-1)          # [N, N]
    out_small = (attn @ K^T)^T                        # [C, N]
    out = feat_wide[b] + bilinear_upsample(out_small.reshape(C,64,64), 4)

HBM traffic is the roofline: ~81 MiB per core (fw 32 + fn center rows 16 +
out 32 + ~1 edge-pair overlap), down from the naive 96. Structure:
  - center rows (4i+1, 4i+2) of fw are cast-DMA'd (f32->bf16, gpsimd SWDGE)
    into a retained SBUF tile rc ONCE; they serve both the Q downsample and
    the residual add later -- avoids the 16 MiB re-read of a naive two-pass.
  - fn center rows cast-DMA'd to bf16 staging, summed to k_bf (2 DVE ops).
  - n-blocks are 256 wide (= one q chunk). Load order q0,q1,k0..k15,q2..q15;
    blocks 0/1 scores are quad-interleaved so the two score-PSUM buffers are
    not held hostage while k streams in, and PV(0) runs standalone right
    after -- earliest possible first y rows / first output stores.
  - K^T (+ ones column for the softmax row-sum) via PE transposes -> kt1;
    PSUM->SBUF copies on ScalarE.
  - exp on ScalarE batched per 4 m-tiles (quarters PE<->Scalar handoffs;
    0.25 ds scale folded into EXP_SCALE), bf16 attn^T tiles.
  - PV accumulates over m; ones column gives the denominator; reciprocal on
    DVE, the normalize-multiply on ScalarE (activation Copy with scale AP).
  - W-upsample via PE matmul with constant block-diag weights (1/4 folded in)
  - H-upsample + residual on DVE in all-bf16 (scalar_tensor_tensor runs at
    ~1 el/ns regardless of dtype -- this ~145 us is the phase-2 pacer);
    edge rows (4k+3, 4k+4) load as adjacent HBM pairs -> 2KB DMA lines;
    stores cast bf16->f32 in the SWDGE DMA.
Known limits: SWDGE keeps only ~4-5 DMAs in flight (~300 GB/s), and the
HW exec time varies +-10% run to run with device clocks.
"""

import math

import numpy as np


# ----------------------------------------------------------------------------
# numpy-side constants
# ----------------------------------------------------------------------------

def _build_upsample_matrix(n_in: int, n_out: int) -> np.ndarray:
    """U[h, H]: out[H] = sum_h U[h, H] * in[h] for torch-style bilinear,
    align_corners=False, antialias=False, scale n_out/n_in."""
    U = np.zeros((n_in, n_out), dtype=np.float64)
    scale = n_in / n_out
    for o in range(n_out):
        src = (o + 0.5) * scale - 0.5
        k0 = int(math.floor(src))
        frac = src - k0
        for k, wt in ((k0, 1.0 - frac), (k0 + 1, frac)):
            kc = min(max(k, 0), n_in - 1)
            U[kc, o] += wt
    return U


def _build_uw_block() -> np.ndarray:
    """[128, 512] block-diag W-upsample weights (two 64->256 blocks), pre-scaled
    by 1/4 to undo the unscaled 2x2-average downsample of K."""
    U = _build_upsample_matrix(64, 256) * 0.25
    blk = np.zeros((128, 512), dtype=np.float64)
    blk[0:64, 0:256] = U
    blk[64:128, 256:512] = U
    return blk


# ----------------------------------------------------------------------------
# Bass kernel builder
# ----------------------------------------------------------------------------

def build_kernel():
    import concourse.bacc as bacc
    import concourse.bass as bass
    import concourse.mybir as mybir
    from concourse import tile

    f32 = mybir.dt.float32
    bf16 = mybir.dt.bfloat16
    AOp = mybir.AluOpType
    ActFn = mybir.ActivationFunctionType

    C = 128          # channels = partitions
    HW = 256         # full resolution
    hw = 64          # downsampled resolution
    N = hw * hw      # 4096 attention positions
    MI = 32          # m tiles of 128
    NBLK = 16        # n blocks of 256 (= one ds q chunk each)
    # scores = (4Q)^T (4K) / (16 sqrt(C)); the ds 2x2 sums are unscaled
    EXP_SCALE = 1.0 / (16.0 * math.sqrt(C))

    nc = bacc.Bacc("TRN2", target_bir_lowering=False, debug=False)

    fw = nc.dram_tensor("feat_wide", [C, HW, HW], f32, kind="ExternalInput")
    fn = nc.dram_tensor("feat_narrow", [C, HW, HW], f32, kind="ExternalInput")
    uw = nc.dram_tensor("uwblk", [128, 512], bf16, kind="ExternalInput")
    ident = nc.dram_tensor("ident", [128, 128], bf16, kind="ExternalInput")
    out = nc.dram_tensor("out", [C, HW, HW], f32, kind="ExternalOutput")

    with tile.TileContext(nc) as tc:
        with (
            tc.tile_pool(name="const", bufs=1) as const_pool,
            tc.tile_pool(name="qk", bufs=1) as qk_pool,
            tc.tile_pool(name="stg", bufs=2) as stg_pool,
            tc.tile_pool(name="rs", bufs=1) as rs_pool,
            tc.tile_pool(name="io", bufs=2) as io_pool,
            tc.tile_pool(name="attn", bufs=2) as attn_pool,
            tc.tile_pool(name="small", bufs=3) as small_pool,
            tc.tile_pool(name="tm", bufs=2) as tm_pool,
            tc.tile_pool(name="ps_s", bufs=2, space=bass.MemorySpace.PSUM) as ps_s,
            tc.tile_pool(name="ps_o", bufs=3, space=bass.MemorySpace.PSUM) as ps_o,
            tc.tile_pool(name="ps_y", bufs=1, space=bass.MemorySpace.PSUM) as ps_y,
        ):
            # ---- constants ----
            uw_t = const_pool.tile([128, 512], bf16)
            nc.sync.dma_start(uw_t[:], uw[:, :])
            id_t = const_pool.tile([128, 128], bf16)
            nc.sync.dma_start(id_t[:], ident[:, :])

            # ---- retained fw center rows (bf16) + q_bf / k_bf [128, 4096] ----
            rc = qk_pool.tile([C, hw, 512], bf16)   # rows 4i+1, 4i+2 of fw
            q_bf = qk_pool.tile([C, N], bf16)
            k_bf = qk_pool.tile([C, N], bf16)
            kt1 = qk_pool.tile([128, MI, 129], bf16)
            nc.vector.memset(kt1[:], 1.0)

            q3 = q_bf[:].rearrange("c (i w) -> c i w", w=hw)
            k3 = k_bf[:].rearrange("c (i w) -> c i w", w=hw)

            DS_I = 4  # i-rows per chunk
            NCH = hw // DS_I

            KI = 8  # k chunks are 2 MiB: the SWDGE ring keeps only ~4-5 DMAs
            # in flight, so doubling the chunk doubles in-flight bytes and the
            # k loads (which gate PV(0) and the first stores) finish earlier

            def emit_k_chunk(cc):
                src3 = fn.ap().rearrange("c (i r) w -> c i (r w)", r=4)
                i0 = cc * KI
                stg = stg_pool.tile([C, KI, 512], bf16, tag="stg")
                # rows 4i+1, 4i+2 are adjacent -> 2KB lines; cast f32->bf16
                nc.gpsimd.dma_start(stg[:], src3[:, i0 : i0 + KI, 256:768])
                rs = rs_pool.tile([C, KI, 256], bf16, tag="rs")
                nc.vector.tensor_tensor(
                    rs[:], stg[:, :, 0:256], stg[:, :, 256:512], AOp.add
                )
                rs4 = rs[:].rearrange("c i (k f) -> c i k f", f=4)
                nc.vector.tensor_tensor(
                    k3[:, i0 : i0 + KI, :], rs4[:, :, :, 1], rs4[:, :, :, 2], AOp.add
                )
                # K^T tiles for this chunk (ones column preset by memset)
                for mi in range(4 * cc, 4 * cc + 4):
                    pt = ps_o.tile([128, 128], bf16, tag="po")
                    nc.tensor.transpose(
                        pt[:], k_bf[:, mi * 128 : (mi + 1) * 128], id_t[:]
                    )
                    nc.scalar.copy(kt1[:, mi, 0:128], pt[:])

            def emit_q_chunk(cc):
                src3 = fw.ap().rearrange("c (i r) w -> c i (r w)", r=4)
                i0 = cc * DS_I
                # cast straight into the retained tile; serves q AND residual
                nc.gpsimd.dma_start(
                    rc[:, i0 : i0 + DS_I, :], src3[:, i0 : i0 + DS_I, 256:768]
                )
                rs = rs_pool.tile([C, DS_I, 256], bf16, tag="rs")
                nc.vector.tensor_tensor(
                    rs[:],
                    rc[:, i0 : i0 + DS_I, 0:256],
                    rc[:, i0 : i0 + DS_I, 256:512],
                    AOp.add,
                )
                rs4 = rs[:].rearrange("c i (k f) -> c i k f", f=4)
                nc.vector.tensor_tensor(
                    q3[:, i0 : i0 + DS_I, :], rs4[:, :, :, 1], rs4[:, :, :, 2], AOp.add
                )

            # one HWDGE ring, strict priority: q0/q1 (block-0/1 scores), all
            # of k (PV needs the full K early), then the rest of q.
            emit_q_chunk(0)
            emit_q_chunk(1)
            for cc in range(hw // KI):
                emit_k_chunk(cc)
            for cc in range(2, NCH):
                emit_q_chunk(cc)

            # ---- y = W-upsampled attention output [128, 64, 256] bf16 ----
            y = qk_pool.tile([C, hw, HW], bf16)
            y3 = y[:]  # [128, 64, 256]

            # ---- attention: interleave scores of block nb with PV of nb-1 ----
            at_tiles = {}

            def emit_scores(nb, mq):
                """scores^T + exp for m-quad mq of n-block nb."""
                at = at_tiles[nb]
                ps = ps_s.tile([128, 4, 256], f32, tag="ps")
                for s in range(4):
                    mi = 4 * mq + s
                    nc.tensor.matmul(
                        ps[:, s, :],
                        k_bf[:, mi * 128 : (mi + 1) * 128],
                        q_bf[:, nb * 256 : (nb + 1) * 256],
                        start=True,
                        stop=True,
                    )
                nc.scalar.activation(
                    at[:, 4 * mq : 4 * mq + 4, :],
                    ps[:],
                    ActFn.Exp,
                    bias=0.0,
                    scale=EXP_SCALE,
                )

            def emit_pv_mms(nb, ns, po, mi0, mi1):
                """PV matmul chunk [mi0, mi1) for n-sub-tile ns of block nb."""
                at = at_tiles[nb]
                for mi in range(mi0, mi1):
                    nc.tensor.matmul(
                        po[:],
                        at[:, mi, ns * 128 : (ns + 1) * 128],
                        kt1[:, mi, :],
                        start=(mi == 0),
                        stop=(mi == MI - 1),
                    )

            def emit_pv_tail(nb, ns, po):
                """normalize + W-up for n-sub-tile ns of n-block nb."""
                t = nb * 2 + ns  # global n-tile (2 h-rows)
                rcp = small_pool.tile([128, 1], f32, tag="rcp")
                nc.vector.reciprocal(rcp[:], po[:, 128:129])
                ot = small_pool.tile([128, 128], bf16, tag="ot")
                nc.scalar.activation(
                    ot[:], po[:, 0:128], ActFn.Copy, bias=0.0, scale=rcp[:]
                )
                py = ps_y.tile([128, 512], f32, tag="py")
                nc.tensor.matmul(py[:], ot[:], uw_t[:], start=True, stop=True)
                nc.scalar.copy(y3[:, 2 * t : 2 * t + 2, :], py[:])

            # blocks 0 and 1 quad-interleaved: both stream at k-chunk pace, so
            # the two ps buffers are not held hostage by block 0 alone and
            # PV(0) can fire the moment the k loads finish.
            at0 = attn_pool.tile([128, MI, 256], bf16, tag="at")
            at1 = attn_pool.tile([128, MI, 256], bf16, tag="at")
            at_tiles[0] = at0
            at_tiles[1] = at1
            for mq in range(8):
                emit_scores(0, mq)
                emit_scores(1, mq)
            po00 = ps_o.tile([128, 129], f32, tag="po")
            emit_pv_mms(0, 0, po00, 0, MI)
            po01 = ps_o.tile([128, 129], f32, tag="po")
            emit_pv_mms(0, 1, po01, 0, MI)
            emit_pv_tail(0, 0, po00)
            emit_pv_tail(0, 1, po01)

            for nb in range(2, NBLK + 1):
                if nb < NBLK:
                    at = attn_pool.tile([128, MI, 256], bf16, tag="at")
                    at_tiles[nb] = at
                # fine interleave: one scores quad (4 MMs + exp), then 8 PV
                # MMs of the previous block -- PV work hides the exp latency.
                pos = {}
                for ns in range(2):
                    po = ps_o.tile([128, 129], f32, tag="po")
                    pos[ns] = po
                    for i in range(4):
                        if nb < NBLK:
                            emit_scores(nb, 4 * ns + i)
                        emit_pv_mms(nb - 1, ns, po, 8 * i, 8 * i + 8)
                emit_pv_tail(nb - 1, 0, pos[0])
                emit_pv_tail(nb - 1, 1, pos[1])

            # ---- H-upsample + residual ----
            # out[4k+r] = wa[r]*y[k+d[r]] + wb[r]*y[k+d[r]+1] + fw[4k+r]
            # center rows (r=1,2) read the retained rc; edge rows (r=0,3) come
            # from pair loads of the adjacent HBM rows (4i+3, 4i+4) -> 2KB
            # lines, cast to bf16. The io tile is all-bf16 (2x DVE mode); the
            # store casts back to f32 in the DMA.
            PH = (
                (0.375, 0.625, -1),
                (0.125, 0.875, -1),
                (0.875, 0.125, 0),
                (0.625, 0.375, 0),
            )
            # mostly KB=8 blocks (amortizes the DVE per-op bubble); the last two
            # are KB=4 so the post-last-y tail is short
            KBLOCKS = [(0, 2)] + [(k, 4) for k in range(2, 62, 4)] + [(62, 2)]
            KBMAX = 4
            fwflat = fw.ap().rearrange("c h w -> c (h w)")
            # pair i = HBM rows (4i+3, 4i+4), i = 0..62, as 2KB lines
            pv = fwflat[:, 768:65280].rearrange("c (i x) -> c i x", x=1024)

            def emit_edge_loads(bi):
                """pp[:, j] = pair k0-1+j = HBM rows (4(k0+j)-1, 4(k0+j));
                row 4k+0 = pp[:, k-k0, 256:512], row 4k+3 = pp[:, k-k0+1, 0:256]."""
                k0, KB = KBLOCKS[bi]
                pp = io_pool.tile([C, KBMAX + 1, 512], bf16, tag="pp")
                if k0 == 0:
                    nc.gpsimd.dma_start(pp[:, 1 : KB + 1, :], pv[:, 0:KB, 0:512])
                    nc.gpsimd.dma_start(pp[:, 0, 256:512], fwflat[:, 0:256])
                elif bi == len(KBLOCKS) - 1:
                    nc.gpsimd.dma_start(pp[:, 0:KB, :], pv[:, k0 - 1 : k0 + KB - 1, 0:512])
                    nc.gpsimd.dma_start(pp[:, KB, 0:256], fwflat[:, 65280:65536])
                else:
                    nc.gpsimd.dma_start(
                        pp[:, 0 : KB + 1, :], pv[:, k0 - 1 : k0 + KB, 0:512]
                    )
                return pp

            rc3 = rc[:]  # [c, 64, 512]: cols 0:256 = row 4i+1, 256:512 = row 4i+2
            pps = {0: emit_edge_loads(0), 1: emit_edge_loads(1)}
            for bi, (k0, KB) in enumerate(KBLOCKS):
                pp = pps.pop(bi)
                ob = io_pool.tile([C, KBMAX, 4, HW], bf16, tag="io")

                def rsrc(r, js, je):
                    if r == 0:
                        return pp[:, js:je, 256:512]
                    if r == 3:
                        return pp[:, js + 1 : je + 1, 0:256]
                    return rc3[:, k0 + js : k0 + je, (r - 1) * 256 : r * 256]

                for r, (wa, wb, d) in enumerate(PH):
                    eng = nc.vector
                    js, je = 0, KB
                    if k0 == 0 and d == -1:
                        js = 1
                    if k0 + KB == hw and d == 0:
                        je = KB - 1
                    # edge rows: clamped -> out = 1.0*y[edge] + fw
                    if js == 1:
                        eng.scalar_tensor_tensor(
                            ob[:, 0, r, :], y3[:, 0, :], 1.0,
                            rsrc(r, 0, 1)[:, 0, :], AOp.mult, AOp.add,
                        )
                    if je == KB - 1:
                        eng.scalar_tensor_tensor(
                            ob[:, KB - 1, r, :], y3[:, hw - 1, :], 1.0,
                            rsrc(r, KB - 1, KB)[:, 0, :], AOp.mult, AOp.add,
                        )
                    cnt = je - js
                    ka = k0 + js + d
                    # ob_rows = wa*y[ka..] + src, then += wb*y[ka+1..] in place
                    eng.scalar_tensor_tensor(
                        ob[:, js:je, r, :], y3[:, ka : ka + cnt, :], wa,
                        rsrc(r, js, je), AOp.mult, AOp.add,
                    )
                    eng.scalar_tensor_tensor(
                        ob[:, js:je, r, :], y3[:, ka + 1 : ka + 1 + cnt, :], wb,
                        ob[:, js:je, r, :], AOp.mult, AOp.add,
                    )
                # cast store bf16 -> f32
                nc.gpsimd.dma_start(
                    out.ap()[:, 4 * k0 : 4 * k0 + 4 * KB, :], ob[:, 0:KB, :, :]
                )
                if bi + 2 < len(KBLOCKS):
                    pps[bi + 2] = emit_edge_loads(bi + 2)

    nc.compile()
    return nc


_NC_CACHE = None


def _get_nc():
    global _NC_CACHE
    if _NC_CACHE is None:
        _NC_CACHE = build_kernel()
    return _NC_CACHE


def run(feat_wide: np.ndarray, feat_narrow: np.ndarray, trace: bool = False):
    """Run on 8 NeuronCores; returns (output [8,128,256,256], BassKernelResults)."""
    from concourse.bass_utils import run_bass_kernel_spmd
    import ml_dtypes

    B, C, H, W = feat_wide.shape
    assert (B, C, H, W) == (8, 128, 256, 256)

    uwblk = _build_uw_block().astype(ml_dtypes.bfloat16)
    identity = np.eye(128, dtype=ml_dtypes.bfloat16)

    nc = _get_nc()
    in_maps = [
        {
            "feat_wide": np.ascontiguousarray(np.asarray(feat_wide[b], dtype=np.float32)),
            "feat_narrow": np.ascontiguousarray(np.asarray(feat_narrow[b], dtype=np.float32)),
            "uwblk": uwblk,
            "ident": identity,
        }
        for b in range(B)
    ]
    res = run_bass_kernel_spmd(nc, in_maps, core_ids=list(range(8)), trace=trace)
    out = np.stack([res.results[b]["out"] for b in range(B)], axis=0)
    return out, res


def kernel(feat_wide: np.ndarray, feat_narrow: np.ndarray) -> np.ndarray:
    out, _ = run(feat_wide, feat_narrow, trace=False)
    return out
